# revision 1
# baseline (speedup 1.0000x reference)
"""MDCT kernel for Trainium2 (8 NeuronCores, batch-parallel).

Math: for frame f (hop 1024, frame len 2048, center-padded), output bin k:
    out[b, f, k] = sum_n xp[b, f*1024 + n] * window[n] * C[n, k]
    C[n, k] = sqrt(2/N) * cos(pi/N * (n + 0.5 + N/2) * (k + 0.5)),  N = 1024

Split the frame into its two hop-aligned rows of X2 = x.reshape(1024, 1024):
    out[f] = Ax[f-1] + Bx[f],   Ax = X2 @ Cw[:1024],  Bx = X2 @ Cw[1024:]
(with Ax[-1] = Bx[1024] = 0 from the center padding).  Both terms accumulate
into the same PSUM tile, so the shift-add costs nothing.

Per-core layout: one batch channel per NeuronCore.  On-chip: X2 is loaded
naturally, transposed via the PE array (fp32 has no DMA transpose) into
X2T[c, rr] with a leading zero column (rr = row + 1), then 16 fp32r matmuls
per 128-frame output tile accumulate A and B parts in PSUM.
"""

import numpy as np

import concourse.bass as bass
import concourse.bacc as bacc
import concourse.mybir as mybir
import concourse.tile as tile
from concourse import masks
from concourse.bass_utils import run_bass_kernel_spmd

B = 8
T = 1 << 20
R = 1024          # rows of X2 per channel (T // hop)
CN = 1024         # row width (hop)
NF = 1025         # output frames
NK = 1024         # output bins
F32 = mybir.dt.float32
F32R = mybir.dt.float32r

_NC_CACHE = None
_CW_CACHE = None


def build_nc() -> bass.Bass:
    nc = bacc.Bacc("TRN2", target_bir_lowering=False, debug=False)
    x = nc.dram_tensor("x", [R, CN], F32, kind="ExternalInput").ap()
    # cw arrives pre-rounded to the fp32r (TF32, 11-bit mantissa) grid, so it
    # can be DMA'd directly as fp32r with no on-chip conversion pass.
    cw = nc.dram_tensor("cw", [2 * CN, NK], F32R, kind="ExternalInput").ap()
    out = nc.dram_tensor("out", [NF, NK], F32, kind="ExternalOutput").ap()

    with tile.TileContext(nc) as tc:
        with (
            tc.tile_pool(name="persist", bufs=1) as persist,
            tc.tile_pool(name="xin", bufs=8) as xin,
            tc.tile_pool(name="outp", bufs=9) as outp,
            tc.tile_pool(name="tps", bufs=3, space="PSUM") as tps,
            tc.tile_pool(name="mmps", bufs=4, space="PSUM") as mmps,
        ):
            # Cw chunks: cwt[p, i, k] = cw[128*i + p, k]; i<8 -> A, i>=8 -> B
            # fp32r operands must be produced by a rounding instruction, so
            # stage the f32 DMA and convert on the vector engine.
            cwt = persist.tile([128, 16, NK], F32R)
            cw_r = cw.rearrange("(i p) k -> p i k", p=128)
            for i in range(16):
                nc.sync.dma_start(cwt[:, i, :], cw_r[:, i, :])

            ident = persist.tile([128, 128], F32)
            masks.make_identity(nc, ident[:])

            # X2T[p, c, rr]: X2 transposed, rr = row + 1, rr=0 is zeros.
            # (memset can't emit f32r, so zero an f32 stage and convert.)
            x2t = persist.tile([128, 8, NF], F32R)
            zstage = persist.tile([128, 8, 1], F32)
            nc.vector.memset(zstage[:], 0.0)
            nc.vector.tensor_copy(x2t[:, :, 0:1], zstage[:])

            def load_and_transpose(i: int):
                xt = xin.tile([128, CN], F32)
                nc.sync.dma_start(xt[:], x[i * 128:(i + 1) * 128, :])
                for c in range(8):
                    pt = tps.tile([128, 128], F32)
                    nc.tensor.transpose(pt[:], xt[:, c * 128:(c + 1) * 128], ident[:])
                    nc.vector.tensor_copy(
                        x2t[:, c, 1 + i * 128:1 + (i + 1) * 128], pt[:]
                    )

            load_and_transpose(0)
            for j in range(8):
                if j < 7:
                    load_and_transpose(j + 1)
                f0 = j * 128
                pa = mmps.tile([128, 512], F32, tag="mm")
                pb = mmps.tile([128, 512], F32, tag="mm")
                for ci in range(16):
                    if ci < 8:
                        # A part: frames f use X2 row f-1  ->  rr = f
                        w = x2t[:, ci, f0:f0 + 128]
                    else:
                        # B part: frames f use X2 row f    ->  rr = f + 1
                        w = x2t[:, ci - 8, f0 + 1:f0 + 129]
                    nc.tensor.matmul(
                        pa[:], w, cwt[:, ci, 0:512],
                        start=(ci == 0), stop=(ci == 15),
                    )
                    nc.tensor.matmul(
                        pb[:], w, cwt[:, ci, 512:1024],
                        start=(ci == 0), stop=(ci == 15),
                    )
                ot = outp.tile([128, NK], F32)
                nc.scalar.copy(ot[:, 0:512], pa[:])
                nc.scalar.copy(ot[:, 512:1024], pb[:])
                nc.sync.dma_start(out[f0:f0 + 128, :], ot[:])

            # Last frame (f = 1024) has only the A part: X2 row 1023 (rr=1024).
            pa = mmps.tile([1, 512], F32, tag="mm")
            pb = mmps.tile([1, 512], F32, tag="mm")
            for c in range(8):
                w = x2t[:, c, 1024:1025]
                nc.tensor.matmul(
                    pa[:], w, cwt[:, c, 0:512],
                    start=(c == 0), stop=(c == 7),
                )
                nc.tensor.matmul(
                    pb[:], w, cwt[:, c, 512:1024],
                    start=(c == 0), stop=(c == 7),
                )
            ot = outp.tile([1, NK], F32, tag="ot_last")
            nc.scalar.copy(ot[:, 0:512], pa[:])
            nc.scalar.copy(ot[:, 512:1024], pb[:])
            nc.sync.dma_start(out[1024:1025, :], ot[:])

    return nc


def make_cw(window: np.ndarray) -> np.ndarray:
    n = np.arange(2 * NK, dtype=np.float64)[:, None]
    k = np.arange(NK, dtype=np.float64)[None, :]
    c = np.sqrt(2.0 / NK) * np.cos(np.pi / NK * (n + 0.5 + NK / 2) * (k + 0.5))
    cw = (window.astype(np.float64)[:, None] * c).astype(np.float32)
    # Round to the fp32r (TF32) grid: RNE to 11 mantissa bits, fp32 layout.
    u = cw.view(np.uint32)
    lsb = (u >> np.uint32(12)) & np.uint32(1)
    u = (u + np.uint32(0x07FF) + lsb) & np.uint32(0xFFFFF000)
    return u.view(np.float32)


def _get_nc() -> bass.Bass:
    global _NC_CACHE
    if _NC_CACHE is None:
        _NC_CACHE = build_nc()
        _NC_CACHE.compile()
    return _NC_CACHE


def run_spmd(x: np.ndarray, window: np.ndarray, **kwargs):
    """Shard, run on 8 cores, return (stacked output, BassKernelResults)."""
    global _CW_CACHE
    if _CW_CACHE is None or _CW_CACHE[0] != window.tobytes():
        _CW_CACHE = (window.tobytes(), make_cw(window))
    cw = _CW_CACHE[1]
    in_maps = [
        {"x": np.ascontiguousarray(x[b].reshape(R, CN)), "cw": cw} for b in range(B)
    ]
    res = run_bass_kernel_spmd(nc=_get_nc(), in_maps=in_maps,
                               core_ids=list(range(B)), **kwargs)
    out = np.stack([res.results[b]["out"] for b in range(B)], axis=0)
    return out, res


def kernel(x: np.ndarray, window: np.ndarray) -> np.ndarray:
    out, _ = run_spmd(np.asarray(x), np.asarray(window))
    return out

